# revision 21
# baseline (speedup 1.0000x reference)
"""Trainium2 Bass kernel for nn_CAModel (neural cellular automata step).

Data-parallel over 8 NeuronCores: 4 images per core.

Per-core layout: each image (16ch x 256x256) is processed as two halves of
128 rows. A half is laid out as [128 partitions, 4610 cols]:
  partition p = 16*q + c  (q = block 0..7 of 16 image rows, c = channel)
  col u = 257 + block_px   (block_px in [0, 4096), flattened row-major;
                            +-257 halo cols hold wrapped neighbor pixels)

v2: PE matmul cost on TRN2 is out-free-size cycles REGARDLESS of K, so the
MLP1 taps (x, y1, y2) are packed into single K=48 matmuls instead of three
accumulating K=32 ones.  A band-stack tile S[128,16384] holds
[x;y1;y2](16 ch each) of block q=2g+P at partitions 64P..64P+48, col block
4096g, filled by SBUF->SBUF DMA permute from the block-layout conv outputs.
MLP1: one matmul (K=48, tile row 64P) per (block, 512px). h psum grouped
[128,1024]; relu+bias evac split between ACT and DVE.  MLP2 and the
mask/tail machinery follow the baseline; pooling moved to GpSimd.
"""

import numpy as np
import ml_dtypes

import concourse.bass as bass
import concourse.mybir as mybir
import concourse.tile as tile
from concourse import bacc

# ---------------- constants ----------------
B, C, H, Wd = 32, 16, 256, 256
PLANE = H * Wd  # 65536
NCORE = 8
NIMG = B // NCORE  # 4 images per core
HALFPX = PLANE // 2  # 32768 px per half (128 rows)
FDH = 4096  # px per block (16 rows)
HALO = 257
XW = HALO + FDH + HALO  # 4610
NJ = FDH // 512  # 8 column chunks per block
ALPHA_CH = 3
ALPHA_THRESH = 0.1
STEP_SIZE = 1.0
HIDDEN = 128

f32 = mybir.dt.float32
bf16 = mybir.dt.bfloat16
i32 = mybir.dt.int32
Alu = mybir.AluOpType
Act = mybir.ActivationFunctionType


def _ap(full: bass.AP, offset_elems: int, dims) -> bass.AP:
    """Build an AP on `full`'s tensor at element offset with explicit dims."""
    return bass.AP(full.tensor, full.offset + offset_elems, [list(d) for d in dims])


def _scols(t: bass.AP, u0: int, step: int, n: int) -> bass.AP:
    """[128, n, 1] AP over strided columns u0 + step*k of a [128, W] tile."""
    full = t[:]
    prow = full.ap[0][0]
    return _ap(full, u0, [[prow, full.ap[0][1]], [step, n], [1, 1]])


def build_kernel(nc: bass.Bass, n_img: int):
    xin = nc.dram_tensor("xin", [n_img, C, PLANE], bf16, kind="ExternalInput")
    fire = nc.dram_tensor("fire", [n_img, PLANE], bf16, kind="ExternalInput")
    wsd = nc.dram_tensor("ws", [128, 128], bf16, kind="ExternalInput")
    wm2d = nc.dram_tensor("wm2", [128, 48], bf16, kind="ExternalInput")
    b1d = nc.dram_tensor("b1t", [128, 1], f32, kind="ExternalInput")
    b2d = nc.dram_tensor("b2t", [128, 1], f32, kind="ExternalInput")
    seld = nc.dram_tensor("selm", [48, 256], bf16, kind="ExternalInput")
    outd = nc.dram_tensor("out", [n_img, C, PLANE], f32, kind="ExternalOutput")

    xin_f = xin.ap()
    fire_f = fire.ap()
    out_f = outd.ap()

    with tile.TileContext(nc) as tc:
        with (
            tc.tile_pool(name="pw", bufs=1) as pw,
            tc.tile_pool(name="pxb", bufs=1) as pxb,
            tc.tile_pool(name="pcs", bufs=2) as pcs,
            tc.tile_pool(name="py", bufs=1) as py,
            tc.tile_pool(name="pst", bufs=1) as pst,
            tc.tile_pool(name="phsb", bufs=2) as phsb,
            tc.tile_pool(name="pdxt", bufs=2) as pdxt,
            tc.tile_pool(name="pout", bufs=1) as pout,
            tc.tile_pool(name="pt1", bufs=1) as pt1,

            tc.tile_pool(name="ppl", bufs=1) as ppl,
            tc.tile_pool(name="psh", bufs=2, space="PSUM") as psh,
            tc.tile_pool(name="psb", bufs=1, space="PSUM") as psb,
            tc.tile_pool(name="psd", bufs=2, space="PSUM") as psd,
        ):
            # ---- weights (once) ----
            ws = pw.tile([128, 128], bf16)
            nc.sync.dma_start(out=ws[:], in_=wsd.ap())
            wm2 = pw.tile([128, 48], bf16)
            nc.sync.dma_start(out=wm2[:], in_=wm2d.ap())
            b1t = pw.tile([128, 1], f32)
            nc.sync.dma_start(out=b1t[:], in_=b1d.ap())
            b2t = pw.tile([128, 1], f32)
            nc.sync.dma_start(out=b2t[:], in_=b2d.ap())
            selm = pw.tile([48, 256], bf16)
            nc.sync.dma_start(out=selm[:], in_=seld.ap())

            for i in range(n_img):
                ibase = i * C * PLANE

                # ============ per-image pool-layout loads ============
                # x3p: alpha plane with +-257 halo; partition p covers
                # px [512p - 257, 512p + 769)
                x3p = ppl.tile([128, 1026], bf16, name="x3p", tag="x3p", bufs=2)
                abase = ibase + ALPHA_CH * PLANE
                nc.sync.dma_start(
                    out=x3p[1:127, :],
                    in_=_ap(xin_f, abase + 512 - 257, [[512, 126], [1, 1026]]),
                )
                nc.sync.dma_start(
                    out=x3p[0:1, 257:1026],
                    in_=_ap(xin_f, abase, [[769, 1], [1, 769]]),
                )
                nc.sync.dma_start(
                    out=x3p[0:1, 0:257],
                    in_=_ap(xin_f, abase + PLANE - 257, [[257, 1], [1, 257]]),
                )
                nc.sync.dma_start(
                    out=x3p[127:128, 0:769],
                    in_=_ap(xin_f, abase + 512 * 127 - 257, [[769, 1], [1, 769]]),
                )
                nc.sync.dma_start(
                    out=x3p[127:128, 769:1026],
                    in_=_ap(xin_f, abase, [[257, 1], [1, 257]]),
                )

                mbf = ppl.tile([128, 512], bf16, name="mbf", tag="mbf", bufs=2)
                nc.sync.dma_start(
                    out=mbf[:], in_=_ap(fire_f, i * PLANE, [[512, 128], [1, 512]])
                )

                # pre-life maxpool on x3p
                pre = self_pool_max(nc, ppl, x3p, "pre", bufs=2)

                dx3p = ppl.tile([128, 512], bf16, name="dx3p", tag="dx3p", bufs=1)
                dxts = []
                xbs = []
                for h in range(2):
                    hbase = ibase + HALFPX * h

                    # ============ load x half (bf16, host-converted) ============
                    xb = pxb.tile([128, XW], bf16, name="xb", tag="xb", bufs=3)
                    xbs.append(xb)
                    nc.sync.dma_start(
                        out=xb[:, HALO : HALO + FDH],
                        in_=_ap(xin_f, hbase, [[FDH, 8], [PLANE, 16], [1, FDH]]),
                    )
                    # left halo
                    if h == 0:
                        nc.sync.dma_start(
                            out=xb[16:128, 0:HALO],
                            in_=_ap(
                                xin_f, ibase + FDH - HALO,
                                [[FDH, 7], [PLANE, 16], [1, HALO]],
                            ),
                        )
                        nc.sync.dma_start(
                            out=xb[0:16, 0:HALO],
                            in_=_ap(xin_f, ibase + PLANE - HALO, [[PLANE, 16], [1, HALO]]),
                        )
                    else:
                        nc.sync.dma_start(
                            out=xb[:, 0:HALO],
                            in_=_ap(
                                xin_f, hbase - HALO,
                                [[FDH, 8], [PLANE, 16], [1, HALO]],
                            ),
                        )
                    # right halo
                    if h == 0:
                        nc.sync.dma_start(
                            out=xb[:, HALO + FDH : XW],
                            in_=_ap(
                                xin_f, hbase + FDH, [[FDH, 8], [PLANE, 16], [1, HALO]]
                            ),
                        )
                    else:
                        nc.sync.dma_start(
                            out=xb[0:112, HALO + FDH : XW],
                            in_=_ap(
                                xin_f, hbase + FDH, [[FDH, 7], [PLANE, 16], [1, HALO]]
                            ),
                        )
                        nc.sync.dma_start(
                            out=xb[112:128, HALO + FDH : XW],
                            in_=_ap(xin_f, ibase, [[PLANE, 16], [1, HALO]]),
                        )

                    # ============ conv (bf16, DVE) ============
                    pt = pcs.tile([128, XW], bf16, name="csA", tag="cs")
                    # p = xb(u+1) - xb(u-1) on [1, 4609)
                    nc.vector.tensor_sub(
                        out=pt[:, 1 : XW - 1], in0=xb[:, 2:XW], in1=xb[:, 0 : XW - 2]
                    )
                    # fix j=0 cols (u = 1 + 256k): p[u] = xb[u+1] - xb[u+255]
                    nfix = (XW - 2 - 1) // 256 + 1  # 18
                    nc.vector.tensor_sub(
                        out=_scols(pt, 1, 256, nfix),
                        in0=_scols(xb, 2, 256, nfix),
                        in1=_scols(xb, 256, 256, nfix),
                    )
                    # fix j=255 cols (u = 256k): p[u] = xb[u-255] - xb[u-1]
                    nc.vector.tensor_sub(
                        out=_scols(pt, 256, 256, nfix),
                        in0=_scols(xb, 1, 256, nfix),
                        in1=_scols(xb, 255, 256, nfix),
                    )
                    # p2 = p + p(+256) on [1, 4353)
                    p2 = pcs.tile([128, XW], bf16, name="csB", tag="cs")
                    nc.vector.tensor_add(
                        out=p2[:, 1 : HALO + FDH],
                        in0=pt[:, 1 : HALO + FDH],
                        in1=pt[:, 257 : HALO + FDH + 256],
                    )
                    # y1 = p2(u) + p2(u-256), valid block px [0, 4096)
                    y1 = py.tile([128, FDH], bf16, name="y1", tag="y1", bufs=2)
                    nc.vector.tensor_add(
                        out=y1[:],
                        in0=p2[:, HALO : HALO + FDH],
                        in1=p2[:, 1 : 1 + FDH],
                    )
                    # s1 = xb(u) + xb(u+1) on [0, 4609)
                    s1 = pcs.tile([128, XW], bf16, name="csC", tag="cs")
                    nc.vector.tensor_add(
                        out=s1[:, 0 : XW - 1], in0=xb[:, 0 : XW - 1], in1=xb[:, 1:XW]
                    )
                    # s2 = s1(u) + s1(u-1) on [1, 4609)
                    s2 = pcs.tile([128, XW], bf16, name="csD", tag="cs")
                    nc.vector.tensor_add(
                        out=s2[:, 1 : XW - 1], in0=s1[:, 1 : XW - 1], in1=s1[:, 0 : XW - 2]
                    )
                    # fix s2 at j=0 (u = 1+256k): s2 = xb[u+255] + 2 xb[u] + xb[u+1]
                    tfx = pcs.tile([128, 32], bf16, name="tfx", tag="tfx", bufs=2)
                    nc.vector.tensor_add(
                        out=_scols(tfx, 0, 1, nfix),
                        in0=_scols(xb, 256, 256, nfix),
                        in1=_scols(xb, 2, 256, nfix),
                    )
                    nc.vector.scalar_tensor_tensor(
                        out=_scols(s2, 1, 256, nfix),
                        in0=_scols(xb, 1, 256, nfix),
                        scalar=2.0,
                        in1=_scols(tfx, 0, 1, nfix),
                        op0=Alu.mult,
                        op1=Alu.add,
                    )
                    # fix s2 at j=255 (u = 256k): s2 = xb[u-255] + 2 xb[u] + xb[u-1]
                    tfx2 = pcs.tile([128, 32], bf16, name="tfx2", tag="tfx", bufs=2)
                    nc.vector.tensor_add(
                        out=_scols(tfx2, 0, 1, nfix),
                        in0=_scols(xb, 1, 256, nfix),
                        in1=_scols(xb, 255, 256, nfix),
                    )
                    nc.vector.scalar_tensor_tensor(
                        out=_scols(s2, 256, 256, nfix),
                        in0=_scols(xb, 256, 256, nfix),
                        scalar=2.0,
                        in1=_scols(tfx2, 0, 1, nfix),
                        op0=Alu.mult,
                        op1=Alu.add,
                    )
                    # y2 = s2(u+256) - s2(u-256), valid block px [0, 4096)
                    y2 = py.tile([128, FDH], bf16, name="y2", tag="y2", bufs=2)
                    nc.vector.tensor_sub(
                        out=y2[:],
                        in0=s2[:, HALO + 256 : HALO + 256 + FDH],
                        in1=s2[:, 1 : 1 + FDH],
                    )

                    # ============ band-stack permute (DMA) ============
                    # S[64P + {0-15,16-31,32-47}, 4096g:+4096] =
                    #   [x; y1; y2] of block q = 2g + P
                    st = pst.tile([128, 16384], bf16, name="st", tag="st")
                    dmaq = [nc.sync, nc.gpsimd, nc.scalar, nc.sync]
                    for q in range(8):
                        P, g = q & 1, q >> 1
                        pb, cg = 64 * P, 4096 * g
                        dmaq[q % 4].dma_start(
                            out=st[pb : pb + 16, cg : cg + FDH],
                            in_=xb[16 * q : 16 * q + 16, HALO : HALO + FDH],
                        )
                        dmaq[(q + 1) % 4].dma_start(
                            out=st[pb + 16 : pb + 32, cg : cg + FDH],
                            in_=y1[16 * q : 16 * q + 16, :],
                        )
                        dmaq[(q + 2) % 4].dma_start(
                            out=st[pb + 32 : pb + 48, cg : cg + FDH],
                            in_=y2[16 * q : 16 * q + 16, :],
                        )

                    # ============ MLP over 1024-px j-groups ============
                    # Software-pipelined: MLP2 matmul pairs of j-group jg-1
                    # are interleaved between MLP1 matmuls of jg so the PE
                    # never waits on the relu evacuations.
                    dxt = pdxt.tile([128, FDH], bf16, name="dxt", tag="dxt")
                    dxts.append(dxt)

                    def mlp2_pair(prev, idx, dxt):
                        jj, s = idx % 2, idx // 2
                        c1 = 1024 * prev["jg"] + 512 * jj
                        key = ("dxps", jj)
                        if key not in prev:
                            prev[key] = psd.tile(
                                [128, 512], f32, name="dxps", tag="dxps"
                            )
                        dxps = prev[key]
                        nc.tensor.matmul(
                            out=dxps[32 * s : 32 * s + 32, :],
                            lhsT=wm2[:, 16:48],
                            rhs=prev[2 * s + 1][:, 512 * jj : 512 * jj + 512],
                            start=True,
                            stop=False,
                            skip_group_check=True,
                            tile_position=(0, 32 * s),
                        )
                        nc.tensor.matmul(
                            out=dxps[32 * s : 32 * s + 16, :],
                            lhsT=wm2[:, 0:16],
                            rhs=prev[2 * s][:, 512 * jj : 512 * jj + 512],
                            start=False,
                            stop=True,
                            skip_group_check=True,
                            tile_position=(0, 32 * s),
                        )
                        if s == 3:
                            nc.scalar.activation(
                                out=dxt[:, c1 : c1 + 512],
                                in_=dxps[:],
                                func=Act.Identity,
                                bias=b2t[:, 0:1],
                                scale=STEP_SIZE,
                            )

                    prev = None
                    for jg in range(5):
                        if jg < 4:
                            cur = {"jg": jg}
                            for q in range(8):
                                P, g = q & 1, q >> 1
                                pb = 64 * P
                                c0 = 4096 * g + 1024 * jg
                                hps = psh.tile(
                                    [128, 1024], f32, name=f"hps{q}", tag="hps"
                                )
                                for k in range(2):
                                    nc.tensor.matmul(
                                        out=hps[:, 512 * k : 512 * k + 512],
                                        lhsT=ws[pb : pb + 48, :],
                                        rhs=st[
                                            pb : pb + 48,
                                            c0 + 512 * k : c0 + 512 * k + 512,
                                        ],
                                        start=True,
                                        stop=True,
                                        tile_position=(pb, 0),
                                    )
                                if prev is not None:
                                    mlp2_pair(prev, q, dxt)
                                hsb = phsb.tile(
                                    [128, 1024], bf16, name=f"hsb{q}", tag=f"hsb{q}"
                                )
                                if q == 1 or (q == 5 and jg % 2 == 0):
                                    nc.vector.tensor_scalar(
                                        out=hsb[:],
                                        in0=hps[:],
                                        scalar1=b1t[:, 0:1],
                                        scalar2=0.0,
                                        op0=Alu.add,
                                        op1=Alu.max,
                                    )
                                else:
                                    nc.scalar.activation(
                                        out=hsb[:],
                                        in_=hps[:],
                                        func=Act.Relu,
                                        bias=b1t[:, 0:1],
                                        scale=1.0,
                                    )
                                cur[q] = hsb
                            prev = cur
                        else:
                            for idx in range(8):
                                mlp2_pair(prev, idx, dxt)

                    # extract dx alpha rows into pool layout
                    # dst partitions 64h+8q+sub <- dxt[3 + 16q, 512*sub + px]
                    for q in range(8):
                        dmaq[q % 4].dma_start(
                            out=dx3p[64 * h + 8 * q : 64 * h + 8 * q + 8, :],
                            in_=_ap(
                                dxt[:], (3 + 16 * q) * FDH,
                                [[FDH, 1], [512, 8], [1, 512]],
                            ),
                        )

                # ============ per-image pooling / masks ============
                tmask = ppl.tile([128, 512], bf16, name="tmask", tag="tmask", bufs=1)
                nc.vector.tensor_mul(out=tmask[:], in0=dx3p[:], in1=mbf[:])
                anp = ppl.tile([128, 1026], bf16, name="anp", tag="anp", bufs=1)
                nc.vector.tensor_add(
                    out=anp[:, 257:769], in0=x3p[:, 257:769], in1=tmask[:]
                )
                # halo gather for anp
                nc.sync.dma_start(out=anp[1:128, 0:257], in_=anp[0:127, 512:769])
                nc.sync.dma_start(out=anp[0:1, 0:257], in_=anp[127:128, 512:769])
                nc.sync.dma_start(out=anp[0:127, 769:1026], in_=anp[1:128, 257:514])
                nc.sync.dma_start(out=anp[127:128, 769:1026], in_=anp[0:1, 257:514])
                post = self_pool_max(nc, ppl, anp, "post")

                nc.vector.tensor_tensor(
                    out=pre[:], in0=pre[:], in1=post[:], op=Alu.min
                )
                life = ppl.tile([128, 512], bf16, name="life", tag="life", bufs=1)
                nc.vector.tensor_scalar(
                    out=life[:], in0=pre[:], scalar1=ALPHA_THRESH, scalar2=None,
                    op0=Alu.is_gt,
                )
                gm = ppl.tile([128, 512], bf16, name="gm", tag="gm", bufs=1)
                nc.vector.tensor_mul(out=gm[:], in0=life[:], in1=mbf[:])

                # compact masks to row-per-block layout: rows 0:16 = life,
                # rows 32:48 = gm (32-aligned for the PE row-tile position)
                lgrow = ppl.tile([48, FDH], bf16, name="lgrow", tag="lgrow")
                nc.sync.dma_start(out=lgrow[0:16, :], in_=life[:])
                nc.sync.dma_start(out=lgrow[32:48, :], in_=gm[:])

                # ============ per-half mask expand (PE) + tail ============
                for h in range(2):
                    xb = xbs[h]
                    dxt = dxts[h]
                    for pi in range(2):
                        out_t = pout.tile([128, 2048], f32, name="ot", tag="ot", bufs=2)
                        for jc in range(4 * pi, 4 * pi + 4):
                            c0 = 512 * jc
                            cl = c0 - 2048 * pi
                            bclp = psb.tile([128, 512], f32, name="bclp", tag="bclp")
                            nc.tensor.matmul(
                                out=bclp[:],
                                lhsT=selm[0:16, 128 * h : 128 * h + 128],
                                rhs=lgrow[0:16, c0 : c0 + 512],
                                start=True,
                                stop=True,
                                tile_position=(0, 0),
                            )
                            bcgp = psb.tile([128, 512], f32, name="bcgp", tag="bcgp")
                            nc.tensor.matmul(
                                out=bcgp[:],
                                lhsT=selm[32:48, 128 * h : 128 * h + 128],
                                rhs=lgrow[32:48, c0 : c0 + 512],
                                start=True,
                                stop=True,
                                tile_position=(32, 0),
                            )
                            t1 = pt1.tile([128, 512], bf16, name="t1", tag="t1", bufs=2)
                            nc.vector.tensor_mul(
                                out=t1[:], in0=dxt[:, c0 : c0 + 512], in1=bcgp[:]
                            )
                            nc.vector.tensor_mul(
                                out=out_t[:, cl : cl + 512],
                                in0=xb[:, HALO + c0 : HALO + c0 + 512],
                                in1=bclp[:],
                            )
                            nc.vector.tensor_add(
                                out=out_t[:, cl : cl + 512],
                                in0=out_t[:, cl : cl + 512],
                                in1=t1[:],
                            )
                        nc.sync.dma_start(
                            out=_ap(
                                out_f, ibase + HALFPX * h + 2048 * pi,
                                [[FDH, 8], [PLANE, 16], [1, 2048]],
                            ),
                            in_=out_t[:],
                        )
    return nc


def self_pool_max(nc, ppl, src, name, bufs=1):
    """3x3 wrap max-pool of a [128, 1026] pool-layout alpha tile.

    Returns [128, 512] tile of pooled values for the valid 512 px.
    Pool layout: partition p covers px [512p - 257, 512p + 769); flat index,
    image col j = (col - 1) mod 256.
    """
    mh = ppl.tile([128, 1026], bf16, name=f"mh_{name}", tag="mh", bufs=2)
    # horizontal 3-max on [1, 1025)
    nc.vector.tensor_tensor(
        out=mh[:, 1:1025], in0=src[:, 0:1024], in1=src[:, 1:1025], op=Alu.max
    )
    nc.vector.tensor_tensor(
        out=mh[:, 1:1025], in0=mh[:, 1:1025], in1=src[:, 2:1026], op=Alu.max
    )
    # fix j=0 cols {1, 257, 513, 769}: max(src[c], src[c+1], src[c+255])
    nc.vector.tensor_max(
        out=_scols(mh, 1, 256, 4), in0=_scols(src, 1, 256, 4), in1=_scols(src, 2, 256, 4)
    )
    nc.vector.tensor_max(
        out=_scols(mh, 1, 256, 4), in0=_scols(mh, 1, 256, 4), in1=_scols(src, 256, 256, 4)
    )
    # fix j=255 cols {256, 512, 768}: max(src[c-1], src[c], src[c-255])
    nc.vector.tensor_max(
        out=_scols(mh, 256, 256, 3),
        in0=_scols(src, 255, 256, 3),
        in1=_scols(src, 256, 256, 3),
    )
    nc.vector.tensor_max(
        out=_scols(mh, 256, 256, 3),
        in0=_scols(mh, 256, 256, 3),
        in1=_scols(src, 1, 256, 3),
    )
    # vertical 3-max -> valid [257, 769)
    out = ppl.tile([128, 512], bf16, name=f"pool_{name}", tag=f"po_{name}", bufs=bufs)
    nc.vector.tensor_tensor(
        out=out[:], in0=mh[:, 1:513], in1=mh[:, 257:769], op=Alu.max
    )
    nc.vector.tensor_tensor(
        out=out[:], in0=out[:], in1=mh[:, 513:1025], op=Alu.max
    )
    return out


def _host_weights(w1, b1, w2, b2):
    w1 = np.asarray(w1, np.float32)
    w2 = np.asarray(w2, np.float32)
    b1 = np.asarray(b1, np.float32)
    b2 = np.asarray(b2, np.float32)
    W1k = [w1[:, 0::3], w1[:, 1::3] * 0.125, w1[:, 2::3] * 0.125]
    ws = np.zeros((128, 128), np.float32)
    for P in range(2):
        for k in range(3):
            ws[64 * P + 16 * k : 64 * P + 16 * k + 16, :] = W1k[k].T
    wm2 = np.zeros((128, 48), np.float32)
    wm2[:, 0:16] = w2.T
    wm2[:, 32:48] = w2.T
    b1t = b1.reshape(128, 1)
    b2t = np.tile(b2 * STEP_SIZE, 8).reshape(128, 1)
    selm = np.zeros((48, 256), np.float32)
    for hh in range(2):
        for q in range(8):
            for c in range(16):
                selm[8 * hh + q, 128 * hh + 16 * q + c] = 1.0
    selm[32:48] = selm[0:16]
    return (
        ws.astype(ml_dtypes.bfloat16),
        wm2.astype(ml_dtypes.bfloat16),
        b1t.astype(np.float32),
        b2t.astype(np.float32),
        selm.astype(ml_dtypes.bfloat16),
    )


_NC_CACHE = {}


def _get_nc(n_img):
    if n_img not in _NC_CACHE:
        nc = bacc.Bacc("TRN2", target_bir_lowering=False, debug=False)
        build_kernel(nc, n_img)
        nc.compile()
        _NC_CACHE[n_img] = nc
    return _NC_CACHE[n_img]


def _host_inputs(x, fire_mask):
    x = np.asarray(x, np.float32).reshape(B, C, PLANE).astype(ml_dtypes.bfloat16)
    fire = (
        np.asarray(fire_mask, np.float32).reshape(B, PLANE).astype(ml_dtypes.bfloat16)
    )
    return x, fire


def kernel(x, w1, b1, w2, b2, fire_mask):
    from concourse.bass_utils import run_bass_kernel_spmd

    x, fire = _host_inputs(x, fire_mask)
    ws, wm2, b1t, b2t, selm = _host_weights(w1, b1, w2, b2)

    nc = _get_nc(NIMG)
    in_maps = []
    for core in range(NCORE):
        sl = slice(core * NIMG, (core + 1) * NIMG)
        in_maps.append(
            {
                "xin": np.ascontiguousarray(x[sl]),
                "fire": np.ascontiguousarray(fire[sl]),
                "ws": ws,
                "wm2": wm2,
                "b1t": b1t,
                "b2t": b2t,
                "selm": selm,
            }
        )
    res = run_bass_kernel_spmd(nc, in_maps, core_ids=list(range(NCORE)))
    outs = [res.results[c]["out"].reshape(NIMG, C, H, Wd) for c in range(NCORE)]
    return np.concatenate(outs, axis=0)



# revision 27
# speedup vs baseline: 1.3860x; 1.3860x over previous
"""Trainium2 Bass kernel for nn_CAModel (neural cellular automata step).

Data-parallel over 8 NeuronCores: 4 images per core.

Per-core layout: each image (16ch x 256x256) is processed as two halves of
128 rows. A half is laid out as [128 partitions, 4610 cols]:
  partition p = 16*q + c  (q = block 0..7 of 16 image rows, c = channel)
  col u = 257 + block_px   (block_px in [0, 4096), flattened row-major;
                            +-257 halo cols hold wrapped neighbor pixels)

v2: PE matmul cost on TRN2 is out-free-size cycles REGARDLESS of K, so the
MLP1 taps (x, y1, y2) are packed into single K=48 matmuls instead of three
accumulating K=32 ones.  A band-stack tile S[128,16384] holds
[x;y1;y2](16 ch each) of block q=2g+P at partitions 64P..64P+48, col block
4096g, filled by SBUF->SBUF DMA permute from the block-layout conv outputs.
MLP1: one matmul (K=48, tile row 64P) per (block, 512px). h psum grouped
[128,1024]; relu+bias evac split between ACT and DVE.  MLP2 and the
mask/tail machinery follow the baseline; pooling moved to GpSimd.
"""

import numpy as np
import ml_dtypes

import concourse.bass as bass
import concourse.mybir as mybir
import concourse.tile as tile
from concourse import bacc

# ---------------- constants ----------------
B, C, H, Wd = 32, 16, 256, 256
PLANE = H * Wd  # 65536
NCORE = 8
NIMG = B // NCORE  # 4 images per core
HALFPX = PLANE // 2  # 32768 px per half (128 rows)
FDH = 4096  # px per block (16 rows)
HALO = 257
XW = HALO + FDH + HALO  # 4610
NJ = FDH // 512  # 8 column chunks per block
ALPHA_CH = 3
ALPHA_THRESH = 0.1
STEP_SIZE = 1.0
HIDDEN = 128

f32 = mybir.dt.float32
bf16 = mybir.dt.bfloat16
i32 = mybir.dt.int32
Alu = mybir.AluOpType
Act = mybir.ActivationFunctionType


def _ap(full: bass.AP, offset_elems: int, dims) -> bass.AP:
    """Build an AP on `full`'s tensor at element offset with explicit dims."""
    return bass.AP(full.tensor, full.offset + offset_elems, [list(d) for d in dims])


def _scols(t: bass.AP, u0: int, step: int, n: int) -> bass.AP:
    """[128, n, 1] AP over strided columns u0 + step*k of a [128, W] tile."""
    full = t[:]
    prow = full.ap[0][0]
    return _ap(full, u0, [[prow, full.ap[0][1]], [step, n], [1, 1]])


def build_kernel(nc: bass.Bass, n_img: int):
    xin = nc.dram_tensor("xin", [n_img, C, PLANE], bf16, kind="ExternalInput")
    fire = nc.dram_tensor("fire", [n_img, PLANE], bf16, kind="ExternalInput")
    wsd = nc.dram_tensor("ws", [128, 128], bf16, kind="ExternalInput")
    wm2d = nc.dram_tensor("wm2", [128, 48], bf16, kind="ExternalInput")
    b1d = nc.dram_tensor("b1t", [128, 1], f32, kind="ExternalInput")
    b2d = nc.dram_tensor("b2t", [128, 1], f32, kind="ExternalInput")
    seld = nc.dram_tensor("selm", [48, 256], bf16, kind="ExternalInput")
    outd = nc.dram_tensor("out", [n_img, C, PLANE], f32, kind="ExternalOutput")

    xin_f = xin.ap()
    fire_f = fire.ap()
    out_f = outd.ap()

    with tile.TileContext(nc) as tc:
        with (
            tc.tile_pool(name="pw", bufs=1) as pw,
            tc.tile_pool(name="pxb", bufs=1) as pxb,
            tc.tile_pool(name="pcs", bufs=2) as pcs,
            tc.tile_pool(name="py", bufs=1) as py,
            tc.tile_pool(name="pst", bufs=1) as pst,
            tc.tile_pool(name="phsb", bufs=2) as phsb,
            tc.tile_pool(name="pdxt", bufs=2) as pdxt,
            tc.tile_pool(name="pout", bufs=1) as pout,
            tc.tile_pool(name="pt1", bufs=1) as pt1,

            tc.tile_pool(name="ppl", bufs=1) as ppl,
            tc.tile_pool(name="psh", bufs=2, space="PSUM") as psh,
            tc.tile_pool(name="psb", bufs=1, space="PSUM") as psb,
            tc.tile_pool(name="psd", bufs=2, space="PSUM") as psd,
        ):
            # ---- weights (once) ----
            ws = pw.tile([128, 128], bf16)
            nc.sync.dma_start(out=ws[:], in_=wsd.ap())
            wm2 = pw.tile([128, 48], bf16)
            nc.sync.dma_start(out=wm2[:], in_=wm2d.ap())
            b1t = pw.tile([128, 1], f32)
            nc.sync.dma_start(out=b1t[:], in_=b1d.ap())
            b2t = pw.tile([128, 1], f32)
            nc.sync.dma_start(out=b2t[:], in_=b2d.ap())
            selm = pw.tile([48, 256], bf16)
            nc.sync.dma_start(out=selm[:], in_=seld.ap())

            for i in range(n_img):
                ibase = i * C * PLANE

                # ============ per-image pool-layout loads ============
                # x3p: alpha plane with +-257 halo; partition p covers
                # px [512p - 257, 512p + 769)
                x3p = ppl.tile([128, 1026], bf16, name="x3p", tag="x3p", bufs=2)
                abase = ibase + ALPHA_CH * PLANE
                nc.sync.dma_start(
                    out=x3p[1:127, :],
                    in_=_ap(xin_f, abase + 512 - 257, [[512, 126], [1, 1026]]),
                )
                nc.sync.dma_start(
                    out=x3p[0:1, 257:1026],
                    in_=_ap(xin_f, abase, [[769, 1], [1, 769]]),
                )
                nc.sync.dma_start(
                    out=x3p[0:1, 0:257],
                    in_=_ap(xin_f, abase + PLANE - 257, [[257, 1], [1, 257]]),
                )
                nc.sync.dma_start(
                    out=x3p[127:128, 0:769],
                    in_=_ap(xin_f, abase + 512 * 127 - 257, [[769, 1], [1, 769]]),
                )
                nc.sync.dma_start(
                    out=x3p[127:128, 769:1026],
                    in_=_ap(xin_f, abase, [[257, 1], [1, 257]]),
                )

                mbf = ppl.tile([128, 512], bf16, name="mbf", tag="mbf", bufs=2)
                nc.sync.dma_start(
                    out=mbf[:], in_=_ap(fire_f, i * PLANE, [[512, 128], [1, 512]])
                )

                # pre-life maxpool on x3p
                pre = self_pool_max(nc, ppl, x3p, "pre", bufs=2)

                dx3p = ppl.tile([128, 512], bf16, name="dx3p", tag="dx3p", bufs=1)
                dxts = []
                xbs = []
                for h in range(2):
                    hbase = ibase + HALFPX * h

                    # ============ load x half (bf16, host-converted) ============
                    xb = pxb.tile([128, XW], bf16, name="xb", tag="xb", bufs=3)
                    xbs.append(xb)
                    nc.sync.dma_start(
                        out=xb[:, HALO : HALO + FDH],
                        in_=_ap(xin_f, hbase, [[FDH, 8], [PLANE, 16], [1, FDH]]),
                    )
                    # left halo
                    if h == 0:
                        nc.sync.dma_start(
                            out=xb[16:128, 0:HALO],
                            in_=_ap(
                                xin_f, ibase + FDH - HALO,
                                [[FDH, 7], [PLANE, 16], [1, HALO]],
                            ),
                        )
                        nc.sync.dma_start(
                            out=xb[0:16, 0:HALO],
                            in_=_ap(xin_f, ibase + PLANE - HALO, [[PLANE, 16], [1, HALO]]),
                        )
                    else:
                        nc.sync.dma_start(
                            out=xb[:, 0:HALO],
                            in_=_ap(
                                xin_f, hbase - HALO,
                                [[FDH, 8], [PLANE, 16], [1, HALO]],
                            ),
                        )
                    # right halo
                    if h == 0:
                        nc.sync.dma_start(
                            out=xb[:, HALO + FDH : XW],
                            in_=_ap(
                                xin_f, hbase + FDH, [[FDH, 8], [PLANE, 16], [1, HALO]]
                            ),
                        )
                    else:
                        nc.sync.dma_start(
                            out=xb[0:112, HALO + FDH : XW],
                            in_=_ap(
                                xin_f, hbase + FDH, [[FDH, 7], [PLANE, 16], [1, HALO]]
                            ),
                        )
                        nc.sync.dma_start(
                            out=xb[112:128, HALO + FDH : XW],
                            in_=_ap(xin_f, ibase, [[PLANE, 16], [1, HALO]]),
                        )

                    # ============ conv (bf16, DVE) ============
                    pt = pcs.tile([128, XW], bf16, name="csA", tag="cs")
                    # p = xb(u+1) - xb(u-1) on [1, 4609)
                    nc.vector.tensor_sub(
                        out=pt[:, 1 : XW - 1], in0=xb[:, 2:XW], in1=xb[:, 0 : XW - 2]
                    )
                    # fix j=0 cols (u = 1 + 256k): p[u] = xb[u+1] - xb[u+255]
                    nfix = (XW - 2 - 1) // 256 + 1  # 18
                    nc.vector.tensor_sub(
                        out=_scols(pt, 1, 256, nfix),
                        in0=_scols(xb, 2, 256, nfix),
                        in1=_scols(xb, 256, 256, nfix),
                    )
                    # fix j=255 cols (u = 256k): p[u] = xb[u-255] - xb[u-1]
                    nc.vector.tensor_sub(
                        out=_scols(pt, 256, 256, nfix),
                        in0=_scols(xb, 1, 256, nfix),
                        in1=_scols(xb, 255, 256, nfix),
                    )
                    # p2 = p + p(+256) on [1, 4353)
                    p2 = pcs.tile([128, XW], bf16, name="csB", tag="cs")
                    nc.vector.tensor_add(
                        out=p2[:, 1 : HALO + FDH],
                        in0=pt[:, 1 : HALO + FDH],
                        in1=pt[:, 257 : HALO + FDH + 256],
                    )
                    # y1 = p2(u) + p2(u-256), valid block px [0, 4096)
                    y1 = py.tile([128, FDH], bf16, name="y1", tag="y1", bufs=2)
                    nc.vector.tensor_add(
                        out=y1[:],
                        in0=p2[:, HALO : HALO + FDH],
                        in1=p2[:, 1 : 1 + FDH],
                    )
                    # s1 = xb(u) + xb(u+1) on [0, 4609)
                    s1 = pcs.tile([128, XW], bf16, name="csC", tag="cs")
                    nc.vector.tensor_add(
                        out=s1[:, 0 : XW - 1], in0=xb[:, 0 : XW - 1], in1=xb[:, 1:XW]
                    )
                    # s2 = s1(u) + s1(u-1) on [1, 4609)
                    s2 = pcs.tile([128, XW], bf16, name="csD", tag="cs")
                    nc.vector.tensor_add(
                        out=s2[:, 1 : XW - 1], in0=s1[:, 1 : XW - 1], in1=s1[:, 0 : XW - 2]
                    )
                    # fix s2 at j=0 (u = 1+256k): s2 = xb[u+255] + 2 xb[u] + xb[u+1]
                    tfx = pcs.tile([128, 32], bf16, name="tfx", tag="tfx", bufs=2)
                    nc.vector.tensor_add(
                        out=_scols(tfx, 0, 1, nfix),
                        in0=_scols(xb, 256, 256, nfix),
                        in1=_scols(xb, 2, 256, nfix),
                    )
                    nc.vector.scalar_tensor_tensor(
                        out=_scols(s2, 1, 256, nfix),
                        in0=_scols(xb, 1, 256, nfix),
                        scalar=2.0,
                        in1=_scols(tfx, 0, 1, nfix),
                        op0=Alu.mult,
                        op1=Alu.add,
                    )
                    # fix s2 at j=255 (u = 256k): s2 = xb[u-255] + 2 xb[u] + xb[u-1]
                    tfx2 = pcs.tile([128, 32], bf16, name="tfx2", tag="tfx", bufs=2)
                    nc.vector.tensor_add(
                        out=_scols(tfx2, 0, 1, nfix),
                        in0=_scols(xb, 1, 256, nfix),
                        in1=_scols(xb, 255, 256, nfix),
                    )
                    nc.vector.scalar_tensor_tensor(
                        out=_scols(s2, 256, 256, nfix),
                        in0=_scols(xb, 256, 256, nfix),
                        scalar=2.0,
                        in1=_scols(tfx2, 0, 1, nfix),
                        op0=Alu.mult,
                        op1=Alu.add,
                    )
                    # y2 = s2(u+256) - s2(u-256), valid block px [0, 4096)
                    y2 = py.tile([128, FDH], bf16, name="y2", tag="y2", bufs=2)
                    nc.vector.tensor_sub(
                        out=y2[:],
                        in0=s2[:, HALO + 256 : HALO + 256 + FDH],
                        in1=s2[:, 1 : 1 + FDH],
                    )

                    # ============ band-stack permute (DMA) ============
                    # S[64P + {0-15,16-31,32-47}, 4096g:+4096] =
                    #   [x; y1; y2] of block q = 2g + P
                    st = pst.tile([128, 16384], bf16, name="st", tag="st")
                    dmaq = [nc.sync, nc.gpsimd, nc.scalar, nc.sync]
                    for q in range(8):
                        P, g = q & 1, q >> 1
                        pb, cg = 64 * P, 4096 * g
                        dmaq[q % 4].dma_start(
                            out=st[pb : pb + 16, cg : cg + FDH],
                            in_=xb[16 * q : 16 * q + 16, HALO : HALO + FDH],
                        )
                        dmaq[(q + 1) % 4].dma_start(
                            out=st[pb + 16 : pb + 32, cg : cg + FDH],
                            in_=y1[16 * q : 16 * q + 16, :],
                        )
                        dmaq[(q + 2) % 4].dma_start(
                            out=st[pb + 32 : pb + 48, cg : cg + FDH],
                            in_=y2[16 * q : 16 * q + 16, :],
                        )

                    # ============ MLP over 1024-px j-groups ============
                    # Software-pipelined: MLP2 matmul pairs of j-group jg-1
                    # are interleaved between MLP1 matmuls of jg so the PE
                    # never waits on the relu evacuations.
                    dxt = pdxt.tile([128, FDH], bf16, name="dxt", tag="dxt")
                    dxts.append(dxt)

                    def mlp2_pair(prev, idx, dxt):
                        jj, s = idx % 2, idx // 2
                        c1 = 1024 * prev["jg"] + 512 * jj
                        key = ("dxps", jj)
                        if key not in prev:
                            prev[key] = psd.tile(
                                [128, 512], f32, name="dxps", tag="dxps"
                            )
                        dxps = prev[key]
                        nc.tensor.matmul(
                            out=dxps[32 * s : 32 * s + 32, :],
                            lhsT=wm2[:, 16:48],
                            rhs=prev[2 * s + 1][:, 512 * jj : 512 * jj + 512],
                            start=True,
                            stop=False,
                            skip_group_check=True,
                            tile_position=(0, 32 * s),
                        )
                        nc.tensor.matmul(
                            out=dxps[32 * s : 32 * s + 16, :],
                            lhsT=wm2[:, 0:16],
                            rhs=prev[2 * s][:, 512 * jj : 512 * jj + 512],
                            start=False,
                            stop=True,
                            skip_group_check=True,
                            tile_position=(0, 32 * s),
                        )
                        if s == 3:
                            nc.scalar.activation(
                                out=dxt[:, c1 : c1 + 512],
                                in_=dxps[:],
                                func=Act.Identity,
                                bias=b2t[:, 0:1],
                                scale=STEP_SIZE,
                            )

                    prev = None
                    for jg in range(5):
                        if jg < 4:
                            cur = {"jg": jg}
                            for q in range(8):
                                P, g = q & 1, q >> 1
                                pb = 64 * P
                                c0 = 4096 * g + 1024 * jg
                                hps = psh.tile(
                                    [128, 1024], f32, name=f"hps{q}", tag="hps"
                                )
                                for k in range(2):
                                    nc.tensor.matmul(
                                        out=hps[:, 512 * k : 512 * k + 512],
                                        lhsT=ws[pb : pb + 48, :],
                                        rhs=st[
                                            pb : pb + 48,
                                            c0 + 512 * k : c0 + 512 * k + 512,
                                        ],
                                        start=True,
                                        stop=True,
                                        tile_position=(pb, 0),
                                    )
                                if prev is not None:
                                    mlp2_pair(prev, q, dxt)
                                hsb = phsb.tile(
                                    [128, 1024], bf16, name=f"hsb{q}", tag=f"hsb{q}"
                                )
                                if q == 1 or (q == 5 and jg % 2 == 0):
                                    nc.vector.tensor_scalar(
                                        out=hsb[:],
                                        in0=hps[:],
                                        scalar1=b1t[:, 0:1],
                                        scalar2=0.0,
                                        op0=Alu.add,
                                        op1=Alu.max,
                                    )
                                else:
                                    nc.scalar.activation(
                                        out=hsb[:],
                                        in_=hps[:],
                                        func=Act.Relu,
                                        bias=b1t[:, 0:1],
                                        scale=1.0,
                                    )
                                cur[q] = hsb
                            prev = cur
                        else:
                            for idx in range(8):
                                mlp2_pair(prev, idx, dxt)

                    # extract dx alpha rows into pool layout
                    # dst partitions 64h+8q+sub <- dxt[3 + 16q, 512*sub + px]
                    for q in range(8):
                        dmaq[q % 4].dma_start(
                            out=dx3p[64 * h + 8 * q : 64 * h + 8 * q + 8, :],
                            in_=_ap(
                                dxt[:], (3 + 16 * q) * FDH,
                                [[FDH, 1], [512, 8], [1, 512]],
                            ),
                        )

                # ============ per-image pooling / masks ============
                tmask = ppl.tile([128, 512], bf16, name="tmask", tag="tmask", bufs=1)
                nc.vector.tensor_mul(out=tmask[:], in0=dx3p[:], in1=mbf[:])
                anp = ppl.tile([128, 1026], bf16, name="anp", tag="anp", bufs=1)
                nc.vector.tensor_add(
                    out=anp[:, 257:769], in0=x3p[:, 257:769], in1=tmask[:]
                )
                # halo gather for anp
                nc.sync.dma_start(out=anp[1:128, 0:257], in_=anp[0:127, 512:769])
                nc.sync.dma_start(out=anp[0:1, 0:257], in_=anp[127:128, 512:769])
                nc.sync.dma_start(out=anp[0:127, 769:1026], in_=anp[1:128, 257:514])
                nc.sync.dma_start(out=anp[127:128, 769:1026], in_=anp[0:1, 257:514])
                post = self_pool_max(nc, ppl, anp, "post")

                nc.vector.tensor_tensor(
                    out=pre[:], in0=pre[:], in1=post[:], op=Alu.min
                )
                life = ppl.tile([128, 512], bf16, name="life", tag="life", bufs=1)
                nc.vector.tensor_scalar(
                    out=life[:], in0=pre[:], scalar1=ALPHA_THRESH, scalar2=None,
                    op0=Alu.is_gt,
                )
                gm = ppl.tile([128, 512], bf16, name="gm", tag="gm", bufs=1)
                nc.vector.tensor_mul(out=gm[:], in0=life[:], in1=mbf[:])

                # compact masks to row-per-block layout: rows 0:16 = life,
                # rows 32:48 = gm (32-aligned for the PE row-tile position)
                lgrow = ppl.tile([48, FDH], bf16, name="lgrow", tag="lgrow")
                nc.sync.dma_start(out=lgrow[0:16, :], in_=life[:])
                nc.sync.dma_start(out=lgrow[32:48, :], in_=gm[:])

                # ============ per-half mask expand (PE) + tail ============
                for h in range(2):
                    xb = xbs[h]
                    dxt = dxts[h]
                    for pi in range(2):
                        out_t = pout.tile([128, 2048], f32, name="ot", tag="ot", bufs=2)
                        for jc in range(4 * pi, 4 * pi + 4):
                            c0 = 512 * jc
                            cl = c0 - 2048 * pi
                            bclp = psb.tile([128, 512], f32, name="bclp", tag="bclp")
                            nc.tensor.matmul(
                                out=bclp[:],
                                lhsT=selm[0:16, 128 * h : 128 * h + 128],
                                rhs=lgrow[0:16, c0 : c0 + 512],
                                start=True,
                                stop=True,
                                tile_position=(0, 0),
                            )
                            bcgp = psb.tile([128, 512], f32, name="bcgp", tag="bcgp")
                            nc.tensor.matmul(
                                out=bcgp[:],
                                lhsT=selm[32:48, 128 * h : 128 * h + 128],
                                rhs=lgrow[32:48, c0 : c0 + 512],
                                start=True,
                                stop=True,
                                tile_position=(32, 0),
                            )
                            t1 = pt1.tile([128, 512], bf16, name="t1", tag="t1", bufs=2)
                            nc.vector.tensor_mul(
                                out=t1[:], in0=dxt[:, c0 : c0 + 512], in1=bcgp[:]
                            )
                            nc.vector.tensor_mul(
                                out=out_t[:, cl : cl + 512],
                                in0=xb[:, HALO + c0 : HALO + c0 + 512],
                                in1=bclp[:],
                            )
                            nc.vector.tensor_add(
                                out=out_t[:, cl : cl + 512],
                                in0=out_t[:, cl : cl + 512],
                                in1=t1[:],
                            )
                        nc.sync.dma_start(
                            out=_ap(
                                out_f, ibase + HALFPX * h + 2048 * pi,
                                [[FDH, 8], [PLANE, 16], [1, 2048]],
                            ),
                            in_=out_t[:],
                        )
    return nc


def self_pool_max(nc, ppl, src, name, bufs=1):
    """3x3 wrap max-pool of a [128, 1026] pool-layout alpha tile.

    Returns [128, 512] tile of pooled values for the valid 512 px.
    Pool layout: partition p covers px [512p - 257, 512p + 769); flat index,
    image col j = (col - 1) mod 256.
    """
    mh = ppl.tile([128, 1026], bf16, name=f"mh_{name}", tag="mh", bufs=2)
    # horizontal 3-max on [1, 1025)
    nc.vector.tensor_tensor(
        out=mh[:, 1:1025], in0=src[:, 0:1024], in1=src[:, 1:1025], op=Alu.max
    )
    nc.vector.tensor_tensor(
        out=mh[:, 1:1025], in0=mh[:, 1:1025], in1=src[:, 2:1026], op=Alu.max
    )
    # fix j=0 cols {1, 257, 513, 769}: max(src[c], src[c+1], src[c+255])
    nc.vector.tensor_max(
        out=_scols(mh, 1, 256, 4), in0=_scols(src, 1, 256, 4), in1=_scols(src, 2, 256, 4)
    )
    nc.vector.tensor_max(
        out=_scols(mh, 1, 256, 4), in0=_scols(mh, 1, 256, 4), in1=_scols(src, 256, 256, 4)
    )
    # fix j=255 cols {256, 512, 768}: max(src[c-1], src[c], src[c-255])
    nc.vector.tensor_max(
        out=_scols(mh, 256, 256, 3),
        in0=_scols(src, 255, 256, 3),
        in1=_scols(src, 256, 256, 3),
    )
    nc.vector.tensor_max(
        out=_scols(mh, 256, 256, 3),
        in0=_scols(mh, 256, 256, 3),
        in1=_scols(src, 1, 256, 3),
    )
    # vertical 3-max -> valid [257, 769)
    out = ppl.tile([128, 512], bf16, name=f"pool_{name}", tag=f"po_{name}", bufs=bufs)
    nc.vector.tensor_tensor(
        out=out[:], in0=mh[:, 1:513], in1=mh[:, 257:769], op=Alu.max
    )
    nc.vector.tensor_tensor(
        out=out[:], in0=out[:], in1=mh[:, 513:1025], op=Alu.max
    )
    return out


def _host_weights(w1, b1, w2, b2):
    w1 = np.asarray(w1, np.float32)
    w2 = np.asarray(w2, np.float32)
    b1 = np.asarray(b1, np.float32)
    b2 = np.asarray(b2, np.float32)
    W1k = [w1[:, 0::3], w1[:, 1::3] * 0.125, w1[:, 2::3] * 0.125]
    ws = np.zeros((128, 128), np.float32)
    for P in range(2):
        for k in range(3):
            ws[64 * P + 16 * k : 64 * P + 16 * k + 16, :] = W1k[k].T
    wm2 = np.zeros((128, 48), np.float32)
    wm2[:, 0:16] = w2.T
    wm2[:, 32:48] = w2.T
    b1t = b1.reshape(128, 1)
    b2t = np.tile(b2 * STEP_SIZE, 8).reshape(128, 1)
    selm = np.zeros((48, 256), np.float32)
    for hh in range(2):
        for q in range(8):
            for c in range(16):
                selm[8 * hh + q, 128 * hh + 16 * q + c] = 1.0
    selm[32:48] = selm[0:16]
    return (
        ws.astype(ml_dtypes.bfloat16),
        wm2.astype(ml_dtypes.bfloat16),
        b1t.astype(np.float32),
        b2t.astype(np.float32),
        selm.astype(ml_dtypes.bfloat16),
    )


_NC_CACHE = {}


def _get_nc(n_img):
    if n_img not in _NC_CACHE:
        nc = bacc.Bacc("TRN2", target_bir_lowering=False, debug=False)
        build_kernel(nc, n_img)
        nc.compile()
        _NC_CACHE[n_img] = nc
    return _NC_CACHE[n_img]


def _host_inputs(x, fire_mask):
    x = np.asarray(x, np.float32).reshape(B, C, PLANE).astype(ml_dtypes.bfloat16)
    fire = (
        np.asarray(fire_mask, np.float32).reshape(B, PLANE).astype(ml_dtypes.bfloat16)
    )
    return x, fire


def kernel(x, w1, b1, w2, b2, fire_mask):
    from concourse.bass_utils import run_bass_kernel_spmd

    x, fire = _host_inputs(x, fire_mask)
    ws, wm2, b1t, b2t, selm = _host_weights(w1, b1, w2, b2)

    nc = _get_nc(NIMG)
    in_maps = []
    for core in range(NCORE):
        sl = slice(core * NIMG, (core + 1) * NIMG)
        in_maps.append(
            {
                "xin": np.ascontiguousarray(x[sl]),
                "fire": np.ascontiguousarray(fire[sl]),
                "ws": ws,
                "wm2": wm2,
                "b1t": b1t,
                "b2t": b2t,
                "selm": selm,
            }
        )
    res = run_bass_kernel_spmd(nc, in_maps, core_ids=list(range(NCORE)))
    outs = [res.results[c]["out"].reshape(NIMG, C, H, Wd) for c in range(NCORE)]
    return np.concatenate(outs, axis=0)



# revision 29
# speedup vs baseline: 1.4973x; 1.0803x over previous
"""Trainium2 Bass kernel for nn_CAModel (neural cellular automata step).

Data-parallel over 8 NeuronCores: 4 images per core.

Per-core layout: each image (16ch x 256x256) is processed as two halves of
128 rows. A half is laid out as [128 partitions, 4610 cols]:
  partition p = 16*q + c  (q = block 0..7 of 16 image rows, c = channel)
  col u = 257 + block_px   (block_px in [0, 4096), flattened row-major;
                            +-257 halo cols hold wrapped neighbor pixels)

v2: PE matmul cost on TRN2 is out-free-size cycles REGARDLESS of K, so the
MLP1 taps (x, y1, y2) are packed into single K=48 matmuls instead of three
accumulating K=32 ones.  A band-stack tile S[128,16384] holds
[x;y1;y2](16 ch each) of block q=2g+P at partitions 64P..64P+48, col block
4096g, filled by SBUF->SBUF DMA permute from the block-layout conv outputs.
MLP1: one matmul (K=48, tile row 64P) per (block, 512px). h psum grouped
[128,1024]; relu+bias evac split between ACT and DVE.  MLP2 and the
mask/tail machinery follow the baseline; pooling moved to GpSimd.
"""

import numpy as np
import ml_dtypes

import concourse.bass as bass
import concourse.mybir as mybir
import concourse.tile as tile
from concourse import bacc

# ---------------- constants ----------------
B, C, H, Wd = 32, 16, 256, 256
PLANE = H * Wd  # 65536
NCORE = 8
NIMG = B // NCORE  # 4 images per core
HALFPX = PLANE // 2  # 32768 px per half (128 rows)
FDH = 4096  # px per block (16 rows)
HALO = 257
XW = HALO + FDH + HALO  # 4610
NJ = FDH // 512  # 8 column chunks per block
ALPHA_CH = 3
ALPHA_THRESH = 0.1
STEP_SIZE = 1.0
HIDDEN = 128

f32 = mybir.dt.float32
bf16 = mybir.dt.bfloat16
i32 = mybir.dt.int32
Alu = mybir.AluOpType
Act = mybir.ActivationFunctionType


def _ap(full: bass.AP, offset_elems: int, dims) -> bass.AP:
    """Build an AP on `full`'s tensor at element offset with explicit dims."""
    return bass.AP(full.tensor, full.offset + offset_elems, [list(d) for d in dims])


def _scols(t: bass.AP, u0: int, step: int, n: int) -> bass.AP:
    """[128, n, 1] AP over strided columns u0 + step*k of a [128, W] tile."""
    full = t[:]
    prow = full.ap[0][0]
    return _ap(full, u0, [[prow, full.ap[0][1]], [step, n], [1, 1]])


def build_kernel(nc: bass.Bass, n_img: int):
    xin = nc.dram_tensor("xin", [n_img, C, PLANE], bf16, kind="ExternalInput")
    fire = nc.dram_tensor("fire", [n_img, PLANE], bf16, kind="ExternalInput")
    wsd = nc.dram_tensor("ws", [128, 128], bf16, kind="ExternalInput")
    wm2d = nc.dram_tensor("wm2", [128, 48], bf16, kind="ExternalInput")
    b1d = nc.dram_tensor("b1t", [128, 1], f32, kind="ExternalInput")
    b2d = nc.dram_tensor("b2t", [128, 1], f32, kind="ExternalInput")
    seld = nc.dram_tensor("selm", [48, 256], bf16, kind="ExternalInput")
    outd = nc.dram_tensor("out", [n_img, C, PLANE], f32, kind="ExternalOutput")

    xin_f = xin.ap()
    fire_f = fire.ap()
    out_f = outd.ap()

    with tile.TileContext(nc) as tc:
        with (
            tc.tile_pool(name="pw", bufs=1) as pw,
            tc.tile_pool(name="pxb", bufs=1) as pxb,
            tc.tile_pool(name="pcs", bufs=2) as pcs,
            tc.tile_pool(name="py", bufs=1) as py,
            tc.tile_pool(name="pst", bufs=1) as pst,
            tc.tile_pool(name="phsb", bufs=2) as phsb,
            tc.tile_pool(name="pdxt", bufs=2) as pdxt,
            tc.tile_pool(name="pout", bufs=1) as pout,
            tc.tile_pool(name="pt1", bufs=1) as pt1,

            tc.tile_pool(name="ppl", bufs=1) as ppl,
            tc.tile_pool(name="psh", bufs=2, space="PSUM") as psh,
            tc.tile_pool(name="psb", bufs=1, space="PSUM") as psb,
            tc.tile_pool(name="psd", bufs=2, space="PSUM") as psd,
        ):
            # ---- weights (once) ----
            ws = pw.tile([128, 128], bf16)
            nc.sync.dma_start(out=ws[:], in_=wsd.ap())
            wm2 = pw.tile([128, 48], bf16)
            nc.sync.dma_start(out=wm2[:], in_=wm2d.ap())
            b1t = pw.tile([128, 1], f32)
            nc.sync.dma_start(out=b1t[:], in_=b1d.ap())
            b2t = pw.tile([128, 1], f32)
            nc.sync.dma_start(out=b2t[:], in_=b2d.ap())
            selm = pw.tile([48, 256], bf16)
            nc.sync.dma_start(out=selm[:], in_=seld.ap())

            for i in range(n_img):
                ibase = i * C * PLANE

                # ============ per-image pool-layout loads ============
                # x3p: alpha plane with +-257 halo; partition p covers
                # px [512p - 257, 512p + 769)
                x3p = ppl.tile([128, 1026], bf16, name="x3p", tag="x3p", bufs=2)
                abase = ibase + ALPHA_CH * PLANE
                nc.sync.dma_start(
                    out=x3p[1:127, :],
                    in_=_ap(xin_f, abase + 512 - 257, [[512, 126], [1, 1026]]),
                )
                nc.sync.dma_start(
                    out=x3p[0:1, 257:1026],
                    in_=_ap(xin_f, abase, [[769, 1], [1, 769]]),
                )
                nc.sync.dma_start(
                    out=x3p[0:1, 0:257],
                    in_=_ap(xin_f, abase + PLANE - 257, [[257, 1], [1, 257]]),
                )
                nc.sync.dma_start(
                    out=x3p[127:128, 0:769],
                    in_=_ap(xin_f, abase + 512 * 127 - 257, [[769, 1], [1, 769]]),
                )
                nc.sync.dma_start(
                    out=x3p[127:128, 769:1026],
                    in_=_ap(xin_f, abase, [[257, 1], [1, 257]]),
                )

                mbf = ppl.tile([128, 512], bf16, name="mbf", tag="mbf", bufs=2)
                nc.sync.dma_start(
                    out=mbf[:], in_=_ap(fire_f, i * PLANE, [[512, 128], [1, 512]])
                )

                # pre-life maxpool on x3p
                pre = self_pool_max(nc, ppl, x3p, "pre", bufs=2)

                dx3p = ppl.tile([128, 512], bf16, name="dx3p", tag="dx3p", bufs=1)
                dxts = []
                xbs = []
                for h in range(2):
                    hbase = ibase + HALFPX * h

                    # ============ load x half (bf16, host-converted) ============
                    xb = pxb.tile([128, XW], bf16, name="xb", tag="xb", bufs=3)
                    xbs.append(xb)
                    nc.sync.dma_start(
                        out=xb[:, HALO : HALO + FDH],
                        in_=_ap(xin_f, hbase, [[FDH, 8], [PLANE, 16], [1, FDH]]),
                    )
                    # left halo
                    if h == 0:
                        nc.sync.dma_start(
                            out=xb[16:128, 0:HALO],
                            in_=_ap(
                                xin_f, ibase + FDH - HALO,
                                [[FDH, 7], [PLANE, 16], [1, HALO]],
                            ),
                        )
                        nc.sync.dma_start(
                            out=xb[0:16, 0:HALO],
                            in_=_ap(xin_f, ibase + PLANE - HALO, [[PLANE, 16], [1, HALO]]),
                        )
                    else:
                        nc.sync.dma_start(
                            out=xb[:, 0:HALO],
                            in_=_ap(
                                xin_f, hbase - HALO,
                                [[FDH, 8], [PLANE, 16], [1, HALO]],
                            ),
                        )
                    # right halo
                    if h == 0:
                        nc.sync.dma_start(
                            out=xb[:, HALO + FDH : XW],
                            in_=_ap(
                                xin_f, hbase + FDH, [[FDH, 8], [PLANE, 16], [1, HALO]]
                            ),
                        )
                    else:
                        nc.sync.dma_start(
                            out=xb[0:112, HALO + FDH : XW],
                            in_=_ap(
                                xin_f, hbase + FDH, [[FDH, 7], [PLANE, 16], [1, HALO]]
                            ),
                        )
                        nc.sync.dma_start(
                            out=xb[112:128, HALO + FDH : XW],
                            in_=_ap(xin_f, ibase, [[PLANE, 16], [1, HALO]]),
                        )

                    # ============ conv (bf16, DVE) ============
                    pt = pcs.tile([128, XW], bf16, name="csA", tag="cs")
                    # p = xb(u+1) - xb(u-1) on [1, 4609)
                    nc.vector.tensor_sub(
                        out=pt[:, 1 : XW - 1], in0=xb[:, 2:XW], in1=xb[:, 0 : XW - 2]
                    )
                    # fix j=0 cols (u = 1 + 256k): p[u] = xb[u+1] - xb[u+255]
                    nfix = (XW - 2 - 1) // 256 + 1  # 18
                    nc.vector.tensor_sub(
                        out=_scols(pt, 1, 256, nfix),
                        in0=_scols(xb, 2, 256, nfix),
                        in1=_scols(xb, 256, 256, nfix),
                    )
                    # fix j=255 cols (u = 256k): p[u] = xb[u-255] - xb[u-1]
                    nc.vector.tensor_sub(
                        out=_scols(pt, 256, 256, nfix),
                        in0=_scols(xb, 1, 256, nfix),
                        in1=_scols(xb, 255, 256, nfix),
                    )
                    # p2 = p + p(+256) on [1, 4353)
                    p2 = pcs.tile([128, XW], bf16, name="csB", tag="cs")
                    nc.vector.tensor_add(
                        out=p2[:, 1 : HALO + FDH],
                        in0=pt[:, 1 : HALO + FDH],
                        in1=pt[:, 257 : HALO + FDH + 256],
                    )
                    # y1 = p2(u) + p2(u-256), valid block px [0, 4096)
                    y1 = py.tile([128, FDH], bf16, name="y1", tag="y1", bufs=2)
                    nc.vector.tensor_add(
                        out=y1[:],
                        in0=p2[:, HALO : HALO + FDH],
                        in1=p2[:, 1 : 1 + FDH],
                    )
                    # s1 = xb(u) + xb(u+1) on [0, 4609)
                    s1 = pcs.tile([128, XW], bf16, name="csC", tag="cs")
                    nc.vector.tensor_add(
                        out=s1[:, 0 : XW - 1], in0=xb[:, 0 : XW - 1], in1=xb[:, 1:XW]
                    )
                    # s2 = s1(u) + s1(u-1) on [1, 4609)
                    s2 = pcs.tile([128, XW], bf16, name="csD", tag="cs")
                    nc.vector.tensor_add(
                        out=s2[:, 1 : XW - 1], in0=s1[:, 1 : XW - 1], in1=s1[:, 0 : XW - 2]
                    )
                    # fix s2 at j=0 (u = 1+256k): s2 = xb[u+255] + 2 xb[u] + xb[u+1]
                    tfx = pcs.tile([128, 32], bf16, name="tfx", tag="tfx", bufs=2)
                    nc.vector.tensor_add(
                        out=_scols(tfx, 0, 1, nfix),
                        in0=_scols(xb, 256, 256, nfix),
                        in1=_scols(xb, 2, 256, nfix),
                    )
                    nc.vector.scalar_tensor_tensor(
                        out=_scols(s2, 1, 256, nfix),
                        in0=_scols(xb, 1, 256, nfix),
                        scalar=2.0,
                        in1=_scols(tfx, 0, 1, nfix),
                        op0=Alu.mult,
                        op1=Alu.add,
                    )
                    # fix s2 at j=255 (u = 256k): s2 = xb[u-255] + 2 xb[u] + xb[u-1]
                    tfx2 = pcs.tile([128, 32], bf16, name="tfx2", tag="tfx", bufs=2)
                    nc.vector.tensor_add(
                        out=_scols(tfx2, 0, 1, nfix),
                        in0=_scols(xb, 1, 256, nfix),
                        in1=_scols(xb, 255, 256, nfix),
                    )
                    nc.vector.scalar_tensor_tensor(
                        out=_scols(s2, 256, 256, nfix),
                        in0=_scols(xb, 256, 256, nfix),
                        scalar=2.0,
                        in1=_scols(tfx2, 0, 1, nfix),
                        op0=Alu.mult,
                        op1=Alu.add,
                    )
                    # y2 = s2(u+256) - s2(u-256), valid block px [0, 4096)
                    y2 = py.tile([128, FDH], bf16, name="y2", tag="y2", bufs=2)
                    nc.vector.tensor_sub(
                        out=y2[:],
                        in0=s2[:, HALO + 256 : HALO + 256 + FDH],
                        in1=s2[:, 1 : 1 + FDH],
                    )

                    # ============ band-stack permute (DMA) ============
                    # S[64P + {0-15,16-31,32-47}, 4096g:+4096] =
                    #   [x; y1; y2] of block q = 2g + P
                    st = pst.tile([128, 16384], bf16, name="st", tag="st")
                    dmaq = [nc.sync, nc.gpsimd, nc.scalar, nc.sync]
                    for q in range(8):
                        P, g = q & 1, q >> 1
                        pb, cg = 64 * P, 4096 * g
                        dmaq[q % 4].dma_start(
                            out=st[pb : pb + 16, cg : cg + FDH],
                            in_=xb[16 * q : 16 * q + 16, HALO : HALO + FDH],
                        )
                        dmaq[(q + 1) % 4].dma_start(
                            out=st[pb + 16 : pb + 32, cg : cg + FDH],
                            in_=y1[16 * q : 16 * q + 16, :],
                        )
                        dmaq[(q + 2) % 4].dma_start(
                            out=st[pb + 32 : pb + 48, cg : cg + FDH],
                            in_=y2[16 * q : 16 * q + 16, :],
                        )

                    # ============ MLP over 1024-px j-groups ============
                    # Software-pipelined: MLP2 matmul pairs of j-group jg-1
                    # are interleaved between MLP1 matmuls of jg so the PE
                    # never waits on the relu evacuations.
                    dxt = pdxt.tile([128, FDH], bf16, name="dxt", tag="dxt")
                    dxts.append(dxt)

                    def mlp2_pair(prev, idx, dxt):
                        jj, s = idx % 2, idx // 2
                        c1 = 1024 * prev["jg"] + 512 * jj
                        key = ("dxps", jj)
                        if key not in prev:
                            prev[key] = psd.tile(
                                [128, 512], f32, name="dxps", tag="dxps"
                            )
                        dxps = prev[key]
                        nc.tensor.matmul(
                            out=dxps[32 * s : 32 * s + 32, :],
                            lhsT=wm2[:, 16:48],
                            rhs=prev[2 * s + 1][:, 512 * jj : 512 * jj + 512],
                            start=True,
                            stop=False,
                            skip_group_check=True,
                            tile_position=(0, 32 * s),
                        )
                        nc.tensor.matmul(
                            out=dxps[32 * s : 32 * s + 16, :],
                            lhsT=wm2[:, 0:16],
                            rhs=prev[2 * s][:, 512 * jj : 512 * jj + 512],
                            start=False,
                            stop=True,
                            skip_group_check=True,
                            tile_position=(0, 32 * s),
                        )
                        if s == 3:
                            nc.scalar.activation(
                                out=dxt[:, c1 : c1 + 512],
                                in_=dxps[:],
                                func=Act.Identity,
                                bias=b2t[:, 0:1],
                                scale=STEP_SIZE,
                            )

                    prev = None
                    for jg in range(5):
                        if jg < 4:
                            cur = {"jg": jg}
                            for q in range(8):
                                P, g = q & 1, q >> 1
                                pb = 64 * P
                                c0 = 4096 * g + 1024 * jg
                                hps = psh.tile(
                                    [128, 1024], f32, name=f"hps{q}", tag="hps"
                                )
                                for k in range(2):
                                    nc.tensor.matmul(
                                        out=hps[:, 512 * k : 512 * k + 512],
                                        lhsT=ws[pb : pb + 48, :],
                                        rhs=st[
                                            pb : pb + 48,
                                            c0 + 512 * k : c0 + 512 * k + 512,
                                        ],
                                        start=True,
                                        stop=True,
                                        tile_position=(pb, 0),
                                    )
                                if prev is not None:
                                    mlp2_pair(prev, q, dxt)
                                hsb = phsb.tile(
                                    [128, 1024], bf16, name=f"hsb{q}", tag=f"hsb{q}"
                                )
                                if q in (1, 5):
                                    nc.vector.tensor_scalar(
                                        out=hsb[:],
                                        in0=hps[:],
                                        scalar1=b1t[:, 0:1],
                                        scalar2=0.0,
                                        op0=Alu.add,
                                        op1=Alu.max,
                                    )
                                else:
                                    nc.scalar.activation(
                                        out=hsb[:],
                                        in_=hps[:],
                                        func=Act.Relu,
                                        bias=b1t[:, 0:1],
                                        scale=1.0,
                                    )
                                cur[q] = hsb
                            prev = cur
                        else:
                            for idx in range(8):
                                mlp2_pair(prev, idx, dxt)

                    # extract dx alpha rows into pool layout
                    # dst partitions 64h+8q+sub <- dxt[3 + 16q, 512*sub + px]
                    for q in range(8):
                        dmaq[q % 4].dma_start(
                            out=dx3p[64 * h + 8 * q : 64 * h + 8 * q + 8, :],
                            in_=_ap(
                                dxt[:], (3 + 16 * q) * FDH,
                                [[FDH, 1], [512, 8], [1, 512]],
                            ),
                        )

                # ============ per-image pooling / masks ============
                tmask = ppl.tile([128, 512], bf16, name="tmask", tag="tmask", bufs=1)
                nc.vector.tensor_mul(out=tmask[:], in0=dx3p[:], in1=mbf[:])
                anp = ppl.tile([128, 1026], bf16, name="anp", tag="anp", bufs=1)
                nc.vector.tensor_add(
                    out=anp[:, 257:769], in0=x3p[:, 257:769], in1=tmask[:]
                )
                # halo gather for anp
                nc.sync.dma_start(out=anp[1:128, 0:257], in_=anp[0:127, 512:769])
                nc.sync.dma_start(out=anp[0:1, 0:257], in_=anp[127:128, 512:769])
                nc.sync.dma_start(out=anp[0:127, 769:1026], in_=anp[1:128, 257:514])
                nc.sync.dma_start(out=anp[127:128, 769:1026], in_=anp[0:1, 257:514])
                post = self_pool_max(nc, ppl, anp, "post")

                nc.vector.tensor_tensor(
                    out=pre[:], in0=pre[:], in1=post[:], op=Alu.min
                )
                life = ppl.tile([128, 512], bf16, name="life", tag="life", bufs=1)
                nc.vector.tensor_scalar(
                    out=life[:], in0=pre[:], scalar1=ALPHA_THRESH, scalar2=None,
                    op0=Alu.is_gt,
                )
                gm = ppl.tile([128, 512], bf16, name="gm", tag="gm", bufs=1)
                nc.vector.tensor_mul(out=gm[:], in0=life[:], in1=mbf[:])

                # compact masks to row-per-block layout: rows 0:16 = life,
                # rows 32:48 = gm (32-aligned for the PE row-tile position)
                lgrow = ppl.tile([48, FDH], bf16, name="lgrow", tag="lgrow")
                nc.sync.dma_start(out=lgrow[0:16, :], in_=life[:])
                nc.sync.dma_start(out=lgrow[32:48, :], in_=gm[:])

                # ============ per-half mask expand (PE) + tail ============
                for h in range(2):
                    xb = xbs[h]
                    dxt = dxts[h]
                    for pi in range(2):
                        out_t = pout.tile([128, 2048], f32, name="ot", tag="ot", bufs=2)
                        for jc in range(4 * pi, 4 * pi + 4):
                            c0 = 512 * jc
                            cl = c0 - 2048 * pi
                            bclp = psb.tile([128, 512], f32, name="bclp", tag="bclp")
                            nc.tensor.matmul(
                                out=bclp[:],
                                lhsT=selm[0:16, 128 * h : 128 * h + 128],
                                rhs=lgrow[0:16, c0 : c0 + 512],
                                start=True,
                                stop=True,
                                tile_position=(0, 0),
                            )
                            bcgp = psb.tile([128, 512], f32, name="bcgp", tag="bcgp")
                            nc.tensor.matmul(
                                out=bcgp[:],
                                lhsT=selm[32:48, 128 * h : 128 * h + 128],
                                rhs=lgrow[32:48, c0 : c0 + 512],
                                start=True,
                                stop=True,
                                tile_position=(32, 0),
                            )
                            t1 = pt1.tile([128, 512], bf16, name="t1", tag="t1", bufs=2)
                            nc.vector.tensor_mul(
                                out=t1[:], in0=dxt[:, c0 : c0 + 512], in1=bcgp[:]
                            )
                            nc.vector.tensor_mul(
                                out=out_t[:, cl : cl + 512],
                                in0=xb[:, HALO + c0 : HALO + c0 + 512],
                                in1=bclp[:],
                            )
                            nc.vector.tensor_add(
                                out=out_t[:, cl : cl + 512],
                                in0=out_t[:, cl : cl + 512],
                                in1=t1[:],
                            )
                        nc.sync.dma_start(
                            out=_ap(
                                out_f, ibase + HALFPX * h + 2048 * pi,
                                [[FDH, 8], [PLANE, 16], [1, 2048]],
                            ),
                            in_=out_t[:],
                        )
    return nc


def self_pool_max(nc, ppl, src, name, bufs=1):
    """3x3 wrap max-pool of a [128, 1026] pool-layout alpha tile.

    Returns [128, 512] tile of pooled values for the valid 512 px.
    Pool layout: partition p covers px [512p - 257, 512p + 769); flat index,
    image col j = (col - 1) mod 256.
    """
    mh = ppl.tile([128, 1026], bf16, name=f"mh_{name}", tag="mh", bufs=2)
    # horizontal 3-max on [1, 1025)
    nc.vector.tensor_tensor(
        out=mh[:, 1:1025], in0=src[:, 0:1024], in1=src[:, 1:1025], op=Alu.max
    )
    nc.vector.tensor_tensor(
        out=mh[:, 1:1025], in0=mh[:, 1:1025], in1=src[:, 2:1026], op=Alu.max
    )
    # fix j=0 cols {1, 257, 513, 769}: max(src[c], src[c+1], src[c+255])
    nc.vector.tensor_max(
        out=_scols(mh, 1, 256, 4), in0=_scols(src, 1, 256, 4), in1=_scols(src, 2, 256, 4)
    )
    nc.vector.tensor_max(
        out=_scols(mh, 1, 256, 4), in0=_scols(mh, 1, 256, 4), in1=_scols(src, 256, 256, 4)
    )
    # fix j=255 cols {256, 512, 768}: max(src[c-1], src[c], src[c-255])
    nc.vector.tensor_max(
        out=_scols(mh, 256, 256, 3),
        in0=_scols(src, 255, 256, 3),
        in1=_scols(src, 256, 256, 3),
    )
    nc.vector.tensor_max(
        out=_scols(mh, 256, 256, 3),
        in0=_scols(mh, 256, 256, 3),
        in1=_scols(src, 1, 256, 3),
    )
    # vertical 3-max -> valid [257, 769)
    out = ppl.tile([128, 512], bf16, name=f"pool_{name}", tag=f"po_{name}", bufs=bufs)
    nc.vector.tensor_tensor(
        out=out[:], in0=mh[:, 1:513], in1=mh[:, 257:769], op=Alu.max
    )
    nc.vector.tensor_tensor(
        out=out[:], in0=out[:], in1=mh[:, 513:1025], op=Alu.max
    )
    return out


def _host_weights(w1, b1, w2, b2):
    w1 = np.asarray(w1, np.float32)
    w2 = np.asarray(w2, np.float32)
    b1 = np.asarray(b1, np.float32)
    b2 = np.asarray(b2, np.float32)
    W1k = [w1[:, 0::3], w1[:, 1::3] * 0.125, w1[:, 2::3] * 0.125]
    ws = np.zeros((128, 128), np.float32)
    for P in range(2):
        for k in range(3):
            ws[64 * P + 16 * k : 64 * P + 16 * k + 16, :] = W1k[k].T
    wm2 = np.zeros((128, 48), np.float32)
    wm2[:, 0:16] = w2.T
    wm2[:, 32:48] = w2.T
    b1t = b1.reshape(128, 1)
    b2t = np.tile(b2 * STEP_SIZE, 8).reshape(128, 1)
    selm = np.zeros((48, 256), np.float32)
    for hh in range(2):
        for q in range(8):
            for c in range(16):
                selm[8 * hh + q, 128 * hh + 16 * q + c] = 1.0
    selm[32:48] = selm[0:16]
    return (
        ws.astype(ml_dtypes.bfloat16),
        wm2.astype(ml_dtypes.bfloat16),
        b1t.astype(np.float32),
        b2t.astype(np.float32),
        selm.astype(ml_dtypes.bfloat16),
    )


_NC_CACHE = {}


def _get_nc(n_img):
    if n_img not in _NC_CACHE:
        nc = bacc.Bacc("TRN2", target_bir_lowering=False, debug=False)
        build_kernel(nc, n_img)
        nc.compile()
        _NC_CACHE[n_img] = nc
    return _NC_CACHE[n_img]


def _host_inputs(x, fire_mask):
    x = np.asarray(x, np.float32).reshape(B, C, PLANE).astype(ml_dtypes.bfloat16)
    fire = (
        np.asarray(fire_mask, np.float32).reshape(B, PLANE).astype(ml_dtypes.bfloat16)
    )
    return x, fire


def kernel(x, w1, b1, w2, b2, fire_mask):
    from concourse.bass_utils import run_bass_kernel_spmd

    x, fire = _host_inputs(x, fire_mask)
    ws, wm2, b1t, b2t, selm = _host_weights(w1, b1, w2, b2)

    nc = _get_nc(NIMG)
    in_maps = []
    for core in range(NCORE):
        sl = slice(core * NIMG, (core + 1) * NIMG)
        in_maps.append(
            {
                "xin": np.ascontiguousarray(x[sl]),
                "fire": np.ascontiguousarray(fire[sl]),
                "ws": ws,
                "wm2": wm2,
                "b1t": b1t,
                "b2t": b2t,
                "selm": selm,
            }
        )
    res = run_bass_kernel_spmd(nc, in_maps, core_ids=list(range(NCORE)))
    outs = [res.results[c]["out"].reshape(NIMG, C, H, Wd) for c in range(NCORE)]
    return np.concatenate(outs, axis=0)

